# revision 21
# baseline (speedup 1.0000x reference)
"""CARAFE++ downsample kernel for Trainium2 (Bass/Tile), 8-way batch-parallel.

Problem (per batch sample, B=8 sharded one-per-core):
  x [128, 160, 160] f32
  compressed = conv1x1(x, w_compress)            -> [16, 160, 160]
  logits     = conv3x3_s2_p1(compressed, w_enc)  -> [25, 80, 80]
  kern       = softmax(logits, axis=0)
  out[c,oh,ow] = sum_{i,j} kern[5i+j,oh,ow] * xpad[c, 2oh+i, 2ow+j]   (pad=2)

Mapping (v2):
  - x is pre-split on the host into zero-padded column-parity planes
    xe/xo (bf16), so every conv/reassembly tap is a stride-1 view and no
    on-chip padding, parity copies or memsets are needed.
  - conv1x1 is folded into the encoder on the host:
    W[o,c,di,dj] = sum_ci w_enc[o,ci,di,dj] * w_comp[ci,c]; the encoder
    becomes nine contraction-128 matmuls straight off xe/xo.
  - softmax normalization happens in the [25, pix] domain (PE ones-matmul
    for the denominator, DVE tensor-tensor divide), so the reassembly
    accumulator in PSUM is already the final output and the store is a
    plain ACT PSUM->SBUF copy per chunk.
  - reassembly per tap: the 25 weight rows are broadcast across the 128
    partitions either by a HWDGE DMA (partition_broadcast view of a DRAM
    stage) or by the Pool engine's partition_broadcast ISA op; DVE does
    the bf16 products (2x mode); the 25-way accumulation runs on the PE
    as identity matmuls in fp32 PSUM.
  - pipelining: output halves are asymmetric (38/42 rows) so half 0's six
    accumulator banks + the encoder's two PSUM banks fit the 8-bank PSUM
    while encoder groups 7..13 are interleaved into half 0's tap stream.
"""

import sys

for p in ("/opt/trn_rl_repo",):
    if p not in sys.path:
        sys.path.insert(0, p)

import numpy as np
import ml_dtypes

import concourse.bass as bass  # noqa: E402
import concourse.mybir as mybir  # noqa: E402
from concourse import bacc  # noqa: E402
from concourse.tile import TileContext  # noqa: E402
from concourse.bass_utils import run_bass_kernel_spmd  # noqa: E402

F32 = mybir.dt.float32
BF16 = mybir.dt.bfloat16
AF = mybir.ActivationFunctionType
ALU = mybir.AluOpType

C = 128          # channels
H = W = 160
HD = WD = 80
K = 5            # reassembly kernel
NT = 25          # K*K
NCORES = 8

ME, MO = 82, 81  # padded parity-plane widths (even / odd source cols)
RT = 164         # padded rows
GR = 6           # encoder output rows per PSUM group (6*80=480 <= 512)
NG = 14          # ceil(80/6) encoder groups

# reassembly bands (output rows): small first band so products start early,
# later bands sized to PSUM (acc chunks + encoder's 2 banks while it overlaps)
BANDS = [(0, 6), (6, 18), (24, 26), (50, 30)]
HMAX = 32

# x row-chunk boundaries (padded R coordinates), ordered so encoder groups
# and band 0 unblock as early as possible
X_CHUNKS = [(0, 20), (20, 38), (38, 62), (62, 98), (98, 131), (131, 164)]

# number of taps per band whose weight-broadcast runs on the Pool engine
# (partition_broadcast ISA op) instead of a HWDGE DMA; more Pool early
# (while the DMA engines still stream x), more DMA late.
POOL_COUNT = [12, 10, 10, 9]


def _enc_schedule():
    """band -> {tap position: encoder group}: groups needed by band b+1
    (beyond those already scheduled) interleave evenly into band b's taps;
    groups for band 0 run before any tap."""
    need_before = []
    for oh0, hr in BANDS:
        need_before.append(-(-(oh0 + hr) // GR))  # groups 0..n-1 staged
    pre = need_before[0]
    sched = {b: {} for b in range(len(BANDS))}
    done = pre
    for b in range(len(BANDS) - 1):
        gs = list(range(done, need_before[b + 1]))
        done = max(done, need_before[b + 1])
        for idx, g in enumerate(gs):
            pos = 1 + (idx * (NT - 4)) // max(len(gs), 1)
            while pos in sched[b]:
                pos += 1
            sched[b][pos] = g
    # any groups left (band boundaries beyond last need): none by design
    return pre, sched


ENC_PRE, ENC_INTERLEAVE = _enc_schedule()


def _reconfig(bands=None, pool_count=None, hmax=None):
    """test-time tuning hook: update module config consistently."""
    global BANDS, POOL_COUNT, HMAX, ENC_PRE, ENC_INTERLEAVE, _NC_CACHE
    if bands is not None:
        BANDS = bands
    if pool_count is not None:
        POOL_COUNT = pool_count
    if hmax is not None:
        HMAX = hmax
    else:
        HMAX = max(hr for _, hr in BANDS)
    ENC_PRE, ENC_INTERLEAVE = _enc_schedule()
    _NC_CACHE = None


def _tap_order(nband):
    """Per-band tap emission order: Pool-broadcast taps (first POOL_COUNT)
    woven evenly among DMA taps, Pool tap first (no DRAM-stage dep), the
    final two taps DMA-type (short drain). Returns [(k, is_pool), ...]."""
    np_, nd = POOL_COUNT[nband], NT - POOL_COUNT[nband]
    pool_ks = [k for k in range(NT) if k % 5 >= 3][:np_]
    pool_ks += [k for k in range(NT) if k % 5 < 3][: np_ - len(pool_ks)]
    dma_ks = [k for k in range(NT) if k not in set(pool_ks)]
    order = []
    ip = idm = 0
    for t in range(NT):
        # fraction scheduling; force DMA for the last two slots
        want_pool = ip < np_ and (
            t < NT - 2 and ip * (NT - 2) <= t * np_
        )
        if want_pool:
            order.append((pool_ks[ip], True))
            ip += 1
        elif idm < nd:
            order.append((dma_ks[idm], False))
            idm += 1
        else:
            order.append((pool_ks[ip], True))
            ip += 1
    return order


def _chunks(n):
    return [(o, min(512, n - o)) for o in range(0, n, 512)]


def _build_bass():
    nc = bacc.Bacc(
        "TRN2",
        target_bir_lowering=False,
        debug=False,
        num_devices=NCORES,
    )

    xe_d = nc.dram_tensor("xe", [C, RT * ME], BF16, kind="ExternalInput").ap()
    xo_d = nc.dram_tensor("xo", [C, RT * MO], BF16, kind="ExternalInput").ap()
    w9T_d = nc.dram_tensor("w9T", [C, 9 * NT], BF16, kind="ExternalInput").ap()
    ones_d = nc.dram_tensor("ones25", [NT, NT], BF16, kind="ExternalInput").ap()
    eye_d = nc.dram_tensor("eye128", [C, C], BF16, kind="ExternalInput").ap()
    out_d = nc.dram_tensor("out", [C, HD, WD], F32, kind="ExternalOutput").ap()
    wt_d = nc.dram_tensor("wt_scratch", [NT, HD * WD], BF16, kind="Internal").ap()
    out2 = out_d.rearrange("c a b -> c (a b)")

    # encoder tap (di,dj): input row r=2oh+di-1 -> stored R=2q+a;
    # input col u=2ow+dj-1 -> parity plane + m offset
    ENC_ROW = {0: (0, 1), 1: (1, 0), 2: (1, 1)}       # di -> (q offset, a)
    ENC_COL = {0: ("xo", 0), 1: ("xe", 1), 2: ("xo", 1)}  # dj -> (plane, m0)

    with TileContext(nc, pool_alloc_mode="queue") as tc:
        with tc.tile_pool(name="persist", bufs=1) as pp:
            xe = pp.tile([C, RT * ME], BF16)
            xo = pp.tile([C, RT * MO], BF16)
            xe3 = xe.rearrange("c (r m) -> c r m", m=ME)
            xo3 = xo.rearrange("c (r m) -> c r m", m=MO)
            xe_d3 = xe_d.rearrange("c (r m) -> c r m", m=ME)
            xo_d3 = xo_d.rearrange("c (r m) -> c r m", m=MO)
            # first row chunk ahead of the (small) weight loads: the encoder
            # can then start at ~4us
            r0, r1 = X_CHUNKS[0]
            nc.sync.dma_start(out=xo3[:, r0:r1, :], in_=xo_d3[:, r0:r1, :])
            nc.sync.dma_start(out=xe3[:, r0:r1, :], in_=xe_d3[:, r0:r1, :])

            w9T = pp.tile([C, 9 * NT], BF16)
            nc.sync.dma_start(out=w9T, in_=w9T_d)
            ones25 = pp.tile([NT, NT], BF16)
            nc.sync.dma_start(out=ones25, in_=ones_d)
            eye = pp.tile([C, C], BF16)
            nc.sync.dma_start(out=eye, in_=eye_d)

            for r0, r1 in X_CHUNKS[1:]:
                nc.sync.dma_start(out=xo3[:, r0:r1, :], in_=xo_d3[:, r0:r1, :])
                nc.sync.dma_start(out=xe3[:, r0:r1, :], in_=xe_d3[:, r0:r1, :])

            wt_sb = pp.tile([NT, HD * WD], BF16)   # normalized softmax weights

            # PE p-state warmup: zero matmuls with no data deps keep the
            # tensor engine's clock ramped through head-phase gaps
            warm_zero = pp.tile([C, 512], BF16)
            nc.vector.memset(warm_zero, 0.0)

            xe4 = xe.rearrange("c (q a m) -> c q a m", a=2, m=ME)
            xo4 = xo.rearrange("c (q a m) -> c q a m", a=2, m=MO)

            def emit_warm(wp, n):
                for _ in range(n):
                    ps = wp.tile([16, 512], F32, tag="warm")
                    nc.tensor.matmul(
                        out=ps, lhsT=warm_zero[:, :16], rhs=warm_zero,
                        start=True, stop=True, skip_group_check=True,
                    )

            def emit_group(pk, pd, ep, rp_, g):
                """encoder conv + softmax for output rows [6g, 6g+nr)."""
                g0 = GR * g
                g1 = min(g0 + GR, HD)
                nr = g1 - g0
                n = nr * WD
                pix = slice(g0 * WD, g1 * WD)
                psk = pk.tile([NT, GR * WD], F32, tag="psk")
                for t, (di, dj) in enumerate(
                    (di, dj) for di in range(3) for dj in range(3)
                ):
                    roff, a = ENC_ROW[di]
                    plane, m0 = ENC_COL[dj]
                    src = xe4 if plane == "xe" else xo4
                    rhs = src[:, g0 + roff : g1 + roff, a, m0 : m0 + WD]
                    nc.tensor.matmul(
                        out=psk[:, :n],
                        lhsT=w9T[:, t * NT : (t + 1) * NT],
                        rhs=rhs,
                        start=t == 0,
                        stop=t == 8,
                    )
                exp_t = ep.tile([NT, GR * WD], BF16, tag="exp")
                nc.scalar.activation(out=exp_t[:, :n], in_=psk[:, :n], func=AF.Exp)
                psd = pd.tile([NT, GR * WD], F32, tag="psd")
                nc.tensor.matmul(
                    out=psd[:, :n], lhsT=ones25, rhs=exp_t[:, :n],
                    start=True, stop=True,
                )
                rcp_t = rp_.tile([NT, GR * WD], F32, tag="rcp")
                nc.vector.reciprocal(out=rcp_t[:, :n], in_=psd[:, :n])
                nc.vector.tensor_mul(
                    out=wt_sb[:, pix], in0=exp_t[:, :n], in1=rcp_t[:, :n]
                )
                nc.scalar.dma_start(out=wt_d[:, pix], in_=wt_sb[:, pix])

            def emit_tap(rp, prp, paccs, oh0, hr, k, is_pool, start, stop):
                """one reassembly tap over output rows [oh0, oh0+hr)."""
                i, j = k // K, k % K
                n = hr * WD
                pix = slice(oh0 * WD, (oh0 + hr) * WD)
                rep = rp.tile([C, HMAX * WD], BF16, tag="rep")
                if is_pool:
                    # Q7 partition_broadcast requires its source at partition
                    # 0: relocate the tap's weight row there first (a single
                    # descriptor SBUF->SBUF DMA, ~20ns on the DMA engines)
                    wtp = wpp.tile([1, HMAX * WD], BF16, tag="wtp")
                    nc.sync.dma_start(out=wtp[:, :n], in_=wt_sb[k : k + 1, pix])
                    nc.gpsimd.partition_broadcast(rep[:, :n], wtp[:, :n])
                else:
                    nc.sync.dma_start(
                        out=rep[:, :n],
                        in_=wt_d[k : k + 1, pix].partition_broadcast(C),
                    )
                src = xe4 if j % 2 == 0 else xo4
                m0 = j // 2 if j % 2 == 0 else (j - 1) // 2
                tap = src[:, oh0 + i // 2 : oh0 + i // 2 + hr, i % 2, m0 : m0 + WD]
                prod = prp.tile([C, HMAX * WD], BF16, tag="prod")
                nc.vector.tensor_mul(out=prod[:, :n], in0=tap, in1=rep[:, :n])
                for c, (o, w) in enumerate(_chunks(n)):
                    nc.tensor.matmul(
                        out=paccs[c][:, :w],
                        lhsT=eye,
                        rhs=prod[:, o : o + w],
                        start=start,
                        stop=stop,
                        skip_group_check=True,
                    )

            def emit_store(op_, paccs, oh0, hr):
                n = hr * WD
                outs = op_.tile([C, HMAX * WD], F32, tag="outs")
                for c, (o, w) in enumerate(_chunks(n)):
                    nc.scalar.copy(out=outs[:, o : o + w], in_=paccs[c][:, :w])
                    nc.sync.dma_start(
                        out=out2[:, oh0 * WD + o : oh0 * WD + o + w],
                        in_=outs[:, o : o + w],
                    )

            with (
                tc.tile_pool(name="rep", bufs=7) as rp,
                tc.tile_pool(name="prod", bufs=5) as prp,
                tc.tile_pool(name="outp", bufs=2) as op_,
                tc.tile_pool(name="warm", bufs=1, space="PSUM") as wp,
                tc.tile_pool(name="wtpart0", bufs=4) as wpp,
            ):
                with (
                    tc.tile_pool(name="psk", bufs=1, space="PSUM") as pk,
                    tc.tile_pool(name="psd", bufs=1, space="PSUM") as pd,
                    tc.tile_pool(name="exp_t", bufs=2) as ep,
                    tc.tile_pool(name="rcp_t", bufs=2) as rp_,
                ):
                    emit_warm(wp, 10)
                    for g in range(ENC_PRE):
                        emit_group(pk, pd, ep, rp_, g)
                        emit_warm(wp, 3)
                    # all but the last band: encoder groups interleaved
                    for b in range(len(BANDS) - 1):
                        oh0, hr = BANDS[b]
                        with tc.tile_pool(
                            name=f"pacc{b}", bufs=1, space="PSUM"
                        ) as pa:
                            paccs = [
                                pa.tile([C, 512], F32, tag=f"pa{b}_{c}",
                                        name=f"pacc{b}_{c}")
                                for c in range(len(_chunks(hr * WD)))
                            ]
                            order = _tap_order(b)
                            for t, (k, is_pool) in enumerate(order):
                                emit_tap(rp, prp, paccs, oh0, hr, k, is_pool,
                                         start=t == 0, stop=t == NT - 1)
                                g = ENC_INTERLEAVE[b].get(t)
                                if g is not None:
                                    emit_group(pk, pd, ep, rp_, g)
                            emit_store(op_, paccs, oh0, hr)
                # last band (encoder pools closed)
                b = len(BANDS) - 1
                oh0, hr = BANDS[b]
                with tc.tile_pool(name=f"pacc{b}", bufs=1, space="PSUM") as pa:
                    paccs = [
                        pa.tile([C, 512], F32, tag=f"pa{b}_{c}",
                                name=f"pacc{b}_{c}")
                        for c in range(len(_chunks(hr * WD)))
                    ]
                    order = _tap_order(b)
                    for t, (k, is_pool) in enumerate(order):
                        emit_tap(rp, prp, paccs, oh0, hr, k, is_pool,
                                 start=t == 0, stop=t == NT - 1)
                    emit_store(op_, paccs, oh0, hr)

    nc.finalize()
    return nc


_NC_CACHE = None


def _get_nc():
    global _NC_CACHE
    if _NC_CACHE is None:
        _NC_CACHE = _build_bass()
    return _NC_CACHE


def _prepare_in_maps(x, w_compress, w_encoder):
    x = np.asarray(x, dtype=np.float32)
    w_compress = np.asarray(w_compress, dtype=np.float32)
    w_encoder = np.asarray(w_encoder, dtype=np.float32)
    B = x.shape[0]
    assert B == NCORES

    bf = ml_dtypes.bfloat16

    # fold conv1x1 into the encoder: W[o,c,di,dj], lhsT layout [c, t*25+o]
    Wf = np.einsum("oikl,ic->ockl", w_encoder, w_compress[:, :, 0, 0])
    w9T = np.ascontiguousarray(
        Wf.transpose(1, 2, 3, 0).reshape(C, 9 * NT)
    ).astype(bf)

    ones = np.ones((NT, NT), dtype=bf)
    eye = np.eye(C, dtype=bf)

    xbf = x.astype(bf)
    # column-parity split with zero padding (pad=2 rows; pad cols in m)
    xe = np.zeros((B, C, RT, ME), dtype=bf)
    xo = np.zeros((B, C, RT, MO), dtype=bf)
    xe[:, :, 2:162, 1:81] = xbf[:, :, :, 0::2]
    xo[:, :, 2:162, 1:81] = xbf[:, :, :, 1::2]

    return [
        {
            "xe": np.ascontiguousarray(xe[b].reshape(C, RT * ME)),
            "xo": np.ascontiguousarray(xo[b].reshape(C, RT * MO)),
            "w9T": w9T,
            "ones25": ones,
            "eye128": eye,
        }
        for b in range(B)
    ]


def kernel(x, w_compress, w_encoder, **run_kwargs):
    in_maps = _prepare_in_maps(x, w_compress, w_encoder)
    nc = _get_nc()
    res = run_bass_kernel_spmd(
        nc, in_maps, core_ids=list(range(NCORES)), **run_kwargs
    )
    out = np.stack([res.results[b]["out"] for b in range(NCORES)], axis=0)
    if run_kwargs:
        kernel.last_results = res
    return out.astype(np.float32)


if __name__ == "__main__":
    rng = np.random.default_rng(0)
    x = rng.standard_normal((8, C, H, W), dtype=np.float32)
    wc = rng.standard_normal((16, C, 1, 1), dtype=np.float32) / np.sqrt(C)
    we = rng.standard_normal((NT, 16, 3, 3), dtype=np.float32) / np.sqrt(16 * 9)
    out = kernel(x, wc, we)
    print(out.shape, out.dtype)


# revision 32
# speedup vs baseline: 1.1699x; 1.1699x over previous
"""CARAFE++ downsample kernel for Trainium2 (Bass/Tile), 8-way batch-parallel.

Problem (per batch sample, B=8 sharded one-per-core):
  x [128, 160, 160] f32
  compressed = conv1x1(x, w_compress)            -> [16, 160, 160]
  logits     = conv3x3_s2_p1(compressed, w_enc)  -> [25, 80, 80]
  kern       = softmax(logits, axis=0)
  out[c,oh,ow] = sum_{i,j} kern[5i+j,oh,ow] * xpad[c, 2oh+i, 2ow+j]   (pad=2)

Mapping (v2):
  - x is pre-split on the host into zero-padded column-parity planes
    xe/xo (bf16), so every conv/reassembly tap is a stride-1 view and no
    on-chip padding, parity copies or memsets are needed.
  - conv1x1 is folded into the encoder on the host:
    W[o,c,di,dj] = sum_ci w_enc[o,ci,di,dj] * w_comp[ci,c]; the encoder
    becomes nine contraction-128 matmuls straight off xe/xo.
  - softmax normalization happens in the [25, pix] domain (PE ones-matmul
    for the denominator, DVE tensor-tensor divide), so the reassembly
    accumulator in PSUM is already the final output and the store is a
    plain ACT PSUM->SBUF copy per chunk.
  - reassembly per tap: the 25 weight rows are broadcast across the 128
    partitions either by a HWDGE DMA (partition_broadcast view of a DRAM
    stage) or by the Pool engine's partition_broadcast ISA op; DVE does
    the bf16 products (2x mode); the 25-way accumulation runs on the PE
    as identity matmuls in fp32 PSUM.
  - pipelining: output halves are asymmetric (38/42 rows) so half 0's six
    accumulator banks + the encoder's two PSUM banks fit the 8-bank PSUM
    while encoder groups 7..13 are interleaved into half 0's tap stream.
"""

import sys

for p in ("/opt/trn_rl_repo",):
    if p not in sys.path:
        sys.path.insert(0, p)

import numpy as np
import ml_dtypes

import concourse.bass as bass  # noqa: E402
import concourse.mybir as mybir  # noqa: E402
from concourse import bacc  # noqa: E402
from concourse.tile import TileContext  # noqa: E402
from concourse.bass_utils import run_bass_kernel_spmd  # noqa: E402

F32 = mybir.dt.float32
BF16 = mybir.dt.bfloat16
AF = mybir.ActivationFunctionType
ALU = mybir.AluOpType

C = 128          # channels
H = W = 160
HD = WD = 80
K = 5            # reassembly kernel
NT = 25          # K*K
NCORES = 8

ME, MO = 82, 81  # padded parity-plane widths (even / odd source cols)
RT = 164         # padded rows
GR = 6           # encoder output rows per PSUM group (6*80=480 <= 512)
NG = 14          # ceil(80/6) encoder groups

# reassembly bands (output rows): small first band so products start early,
# later bands sized to PSUM (acc chunks + encoder's 2 banks while it overlaps)
BANDS = [(0, 8), (8, 18), (26, 24), (50, 30)]
HMAX = 32

# x row-chunk boundaries (padded R coordinates), ordered so encoder groups
# and band 0 unblock as early as possible
X_CHUNKS = [(0, 20), (20, 38), (38, 62), (62, 98), (98, 131), (131, 164)]

# number of taps per band whose weight-broadcast runs on the Pool engine
# (partition_broadcast ISA op) instead of a HWDGE DMA; more Pool early
# (while the DMA engines still stream x), more DMA late.
POOL_COUNT = [12, 10, 9, 8]


def _enc_schedule():
    """band -> {tap position: encoder group}: groups needed by band b+1
    (beyond those already scheduled) interleave evenly into band b's taps;
    groups for band 0 run before any tap."""
    pre = -(-(BANDS[0][0] + BANDS[0][1]) // GR)
    sched = {b: {} for b in range(len(BANDS))}
    # groups run as early as PSUM allows: up to 7 more during band 0, the
    # rest during band 1 (encoder psum pools close before band 2)
    caps = {0: min(NG, pre + 7), 1: NG}
    done = pre
    for b in (0, 1):
        for idx, g in enumerate(range(done, caps[b])):
            sched[b][1 + idx] = g
        done = caps[b]
    return pre, sched


ENC_PRE, ENC_INTERLEAVE = _enc_schedule()


def _reconfig(bands=None, pool_count=None, hmax=None):
    """test-time tuning hook: update module config consistently."""
    global BANDS, POOL_COUNT, HMAX, ENC_PRE, ENC_INTERLEAVE, _NC_CACHE
    if bands is not None:
        BANDS = bands
    if pool_count is not None:
        POOL_COUNT = pool_count
    if hmax is not None:
        HMAX = hmax
    else:
        HMAX = max(hr for _, hr in BANDS)
    ENC_PRE, ENC_INTERLEAVE = _enc_schedule()
    _NC_CACHE = None


def _tap_order(nband):
    """Per-band tap emission order: Pool-broadcast taps (first POOL_COUNT)
    woven evenly among DMA taps, Pool tap first (no DRAM-stage dep), the
    final two taps DMA-type (short drain). Returns [(k, is_pool), ...]."""
    np_, nd = POOL_COUNT[nband], NT - POOL_COUNT[nband]
    pool_ks = [0] + [k for k in range(NT) if k % 5 >= 3][: np_ - 1]
    pool_ks += [k for k in range(1, NT) if k % 5 < 3][: np_ - len(pool_ks)]
    dma_ks = [k for k in range(NT) if k not in set(pool_ks)]
    order = []
    ip = idm = 0
    for t in range(NT):
        # fraction scheduling; force DMA for the last two slots
        want_pool = ip < np_ and (
            t < NT - 2 and ip * (NT - 2) <= t * np_
        )
        if want_pool:
            order.append((pool_ks[ip], True))
            ip += 1
        elif idm < nd:
            order.append((dma_ks[idm], False))
            idm += 1
        else:
            order.append((pool_ks[ip], True))
            ip += 1
    return order


def _chunks(n):
    return [(o, min(512, n - o)) for o in range(0, n, 512)]


def _build_bass():
    nc = bacc.Bacc(
        "TRN2",
        target_bir_lowering=False,
        debug=False,
        num_devices=NCORES,
    )

    xe_d = nc.dram_tensor("xe", [C, RT * ME], BF16, kind="ExternalInput").ap()
    xo_d = nc.dram_tensor("xo", [C, RT * MO], BF16, kind="ExternalInput").ap()
    w9T_d = nc.dram_tensor("w9T", [C, 9 * NT], BF16, kind="ExternalInput").ap()
    ones_d = nc.dram_tensor("ones25", [NT, NT], BF16, kind="ExternalInput").ap()
    eye_d = nc.dram_tensor("eye128", [C, C], BF16, kind="ExternalInput").ap()
    out_d = nc.dram_tensor("out", [C, HD, WD], F32, kind="ExternalOutput").ap()
    wt_d = nc.dram_tensor("wt_scratch", [NT, HD * WD], BF16, kind="Internal").ap()
    out2 = out_d.rearrange("c a b -> c (a b)")

    # encoder tap (di,dj): input row r=2oh+di-1 -> stored R=2q+a;
    # input col u=2ow+dj-1 -> parity plane + m offset
    ENC_ROW = {0: (0, 1), 1: (1, 0), 2: (1, 1)}       # di -> (q offset, a)
    ENC_COL = {0: ("xo", 0), 1: ("xe", 1), 2: ("xo", 1)}  # dj -> (plane, m0)

    with TileContext(nc, pool_alloc_mode="queue") as tc:
        with tc.tile_pool(name="persist", bufs=1) as pp:
            xe = pp.tile([C, RT * ME], BF16)
            xo = pp.tile([C, RT * MO], BF16)
            xe3 = xe.rearrange("c (r m) -> c r m", m=ME)
            xo3 = xo.rearrange("c (r m) -> c r m", m=MO)
            xe_d3 = xe_d.rearrange("c (r m) -> c r m", m=ME)
            xo_d3 = xo_d.rearrange("c (r m) -> c r m", m=MO)
            # first row chunk ahead of the (small) weight loads: the encoder
            # can then start at ~4us
            r0, r1 = X_CHUNKS[0]
            nc.sync.dma_start(out=xo3[:, r0:r1, :], in_=xo_d3[:, r0:r1, :])
            nc.sync.dma_start(out=xe3[:, r0:r1, :], in_=xe_d3[:, r0:r1, :])

            w9T = pp.tile([C, 9 * NT], BF16)
            nc.sync.dma_start(out=w9T, in_=w9T_d)
            ones25 = pp.tile([NT, NT], BF16)
            nc.sync.dma_start(out=ones25, in_=ones_d)
            eye = pp.tile([C, C], BF16)
            nc.sync.dma_start(out=eye, in_=eye_d)

            for r0, r1 in X_CHUNKS[1:3]:
                nc.sync.dma_start(out=xo3[:, r0:r1, :], in_=xo_d3[:, r0:r1, :])
                nc.sync.dma_start(out=xe3[:, r0:r1, :], in_=xe_d3[:, r0:r1, :])

            def emit_xchunk(ci):
                r0, r1 = X_CHUNKS[ci]
                nc.sync.dma_start(out=xo3[:, r0:r1, :], in_=xo_d3[:, r0:r1, :])
                nc.sync.dma_start(out=xe3[:, r0:r1, :], in_=xe_d3[:, r0:r1, :])

            wt_sb = pp.tile([NT, HD * WD], BF16)   # normalized softmax weights

            # PE p-state warmup: zero matmuls with no data deps keep the
            # tensor engine's clock ramped through head-phase gaps
            warm_zero = pp.tile([C, 512], BF16)
            nc.vector.memset(warm_zero, 0.0)

            xe4 = xe.rearrange("c (q a m) -> c q a m", a=2, m=ME)
            xo4 = xo.rearrange("c (q a m) -> c q a m", a=2, m=MO)

            def emit_warm(wp, n):
                for _ in range(n):
                    ps = wp.tile([16, 512], F32, tag="warm")
                    nc.tensor.matmul(
                        out=ps, lhsT=warm_zero[:, :16], rhs=warm_zero,
                        start=True, stop=True, skip_group_check=True,
                    )

            def emit_group(pk, pd, ep, rp_, g):
                """encoder conv + softmax for output rows [6g, 6g+nr)."""
                g0 = GR * g
                g1 = min(g0 + GR, HD)
                nr = g1 - g0
                n = nr * WD
                pix = slice(g0 * WD, g1 * WD)
                psk = pk.tile([NT, GR * WD], F32, tag="psk")
                for t, (di, dj) in enumerate(
                    (di, dj) for di in range(3) for dj in range(3)
                ):
                    roff, a = ENC_ROW[di]
                    plane, m0 = ENC_COL[dj]
                    src = xe4 if plane == "xe" else xo4
                    rhs = src[:, g0 + roff : g1 + roff, a, m0 : m0 + WD]
                    nc.tensor.matmul(
                        out=psk[:, :n],
                        lhsT=w9T[:, t * NT : (t + 1) * NT],
                        rhs=rhs,
                        start=t == 0,
                        stop=t == 8,
                    )
                exp_t = ep.tile([NT, GR * WD], BF16, tag="exp")
                nc.scalar.activation(out=exp_t[:, :n], in_=psk[:, :n], func=AF.Exp)
                psd = pd.tile([NT, GR * WD], F32, tag="psd")
                nc.tensor.matmul(
                    out=psd[:, :n], lhsT=ones25, rhs=exp_t[:, :n],
                    start=True, stop=True,
                )
                rcp_t = rp_.tile([NT, GR * WD], BF16, tag="rcp")
                with nc.allow_low_precision("softmax denominator recip in bf16"):
                    nc.vector.reciprocal(out=rcp_t[:, :n], in_=psd[:, :n])
                nc.vector.tensor_mul(
                    out=wt_sb[:, pix], in0=exp_t[:, :n], in1=rcp_t[:, :n]
                )
                nc.scalar.dma_start(out=wt_d[:, pix], in_=wt_sb[:, pix])

            def emit_relocate(oh0, hr, k):
                # Q7 partition_broadcast requires its source at partition 0:
                # tap 0 already lives there; other taps' weight rows are
                # relocated ahead of time (a single-descriptor SBUF->SBUF
                # DMA, ~20ns on the DMA engines)
                n = hr * WD
                pix = slice(oh0 * WD, (oh0 + hr) * WD)
                if k == 0:
                    return wt_sb[0:1, pix]
                wtp = wpp.tile([1, HMAX * WD], BF16, tag="wtp")
                nc.sync.dma_start(out=wtp[:, :n], in_=wt_sb[k : k + 1, pix])
                return wtp

            def emit_tap(rp, prp, paccs, oh0, hr, k, wtp, start, stop):
                """one reassembly tap over output rows [oh0, oh0+hr)."""
                i, j = k // K, k % K
                n = hr * WD
                pix = slice(oh0 * WD, (oh0 + hr) * WD)
                rep = rp.tile([C, HMAX * WD], BF16, tag="rep")
                if wtp is not None:
                    nc.gpsimd.partition_broadcast(rep[:, :n], wtp[:, :n])
                else:
                    nc.sync.dma_start(
                        out=rep[:, :n],
                        in_=wt_d[k : k + 1, pix].partition_broadcast(C),
                    )
                src = xe4 if j % 2 == 0 else xo4
                m0 = j // 2 if j % 2 == 0 else (j - 1) // 2
                tap = src[:, oh0 + i // 2 : oh0 + i // 2 + hr, i % 2, m0 : m0 + WD]
                prod = prp.tile([C, HMAX * WD], BF16, tag="prod")
                nc.vector.tensor_mul(out=prod[:, :n], in0=tap, in1=rep[:, :n])
                for c, (o, w) in enumerate(_chunks(n)):
                    nc.tensor.matmul(
                        out=paccs[c][:, :w],
                        lhsT=eye,
                        rhs=prod[:, o : o + w],
                        start=start,
                        stop=stop,
                        skip_group_check=True,
                    )

            def emit_store(op_, paccs, oh0, hr):
                n = hr * WD
                outs = op_.tile([C, HMAX * WD], F32, tag="outs")
                chunks = _chunks(n)
                for c, (o, w) in enumerate(chunks):
                    nc.scalar.copy(out=outs[:, o : o + w], in_=paccs[c][:, :w])
                for c, (o, w) in enumerate(chunks):
                    nc.scalar.dma_start(
                        out=out2[:, oh0 * WD + o : oh0 * WD + o + w],
                        in_=outs[:, o : o + w],
                    )

            with (
                tc.tile_pool(name="rep", bufs=7) as rp,
                tc.tile_pool(name="prod", bufs=5) as prp,
                tc.tile_pool(name="outp", bufs=2) as op_,
                tc.tile_pool(name="wtpart0", bufs=5) as wpp,
            ):
                with (
                    tc.tile_pool(name="psk", bufs=2, space="PSUM") as pk,
                    tc.tile_pool(name="psd", bufs=2, space="PSUM") as pd,
                    tc.tile_pool(name="exp_t", bufs=2) as ep,
                    tc.tile_pool(name="rcp_t", bufs=2) as rp_,
                ):
                    with tc.tile_pool(name="warm", bufs=1, space="PSUM") as wp:
                        emit_warm(wp, 10)
                        for g in range(ENC_PRE):
                            emit_group(pk, pd, ep, rp_, g)
                            emit_warm(wp, 3)
                    # bands 0..1 host the interleaved encoder groups
                    for b in range(2):
                        oh0, hr = BANDS[b]
                        with tc.tile_pool(
                            name=f"pacc{b}", bufs=1, space="PSUM"
                        ) as pa:
                            paccs = [
                                pa.tile([C, 512], F32, tag=f"pa{b}_{c}",
                                        name=f"pacc{b}_{c}")
                                for c in range(len(_chunks(hr * WD)))
                            ]
                            order = _tap_order(b)
                            wtps = {
                                t: emit_relocate(oh0, hr, kk)
                                for t, (kk, isp) in enumerate(order[:2]) if isp
                            }
                            for t, (k, is_pool) in enumerate(order):
                                if b == 0 and t in (1, 4, 7):
                                    emit_xchunk(3 + (t - 1) // 3)
                                if t + 2 < NT and order[t + 2][1]:
                                    wtps[t + 2] = emit_relocate(
                                        oh0, hr, order[t + 2][0])
                                emit_tap(rp, prp, paccs, oh0, hr, k,
                                         wtps.pop(t, None),
                                         start=t == 0, stop=t == NT - 1)
                                g = ENC_INTERLEAVE[b].get(t)
                                if g is not None:
                                    emit_group(pk, pd, ep, rp_, g)
                            emit_store(op_, paccs, oh0, hr)
                # later bands (encoder pools closed)
                for b in range(2, len(BANDS)):
                    oh0, hr = BANDS[b]
                    with tc.tile_pool(name=f"pacc{b}", bufs=1, space="PSUM") as pa:
                        paccs = [
                            pa.tile([C, 512], F32, tag=f"pa{b}_{c}",
                                    name=f"pacc{b}_{c}")
                            for c in range(len(_chunks(hr * WD)))
                        ]
                        order = _tap_order(b)
                        wtps = {
                            t: emit_relocate(oh0, hr, kk)
                            for t, (kk, isp) in enumerate(order[:2]) if isp
                        }
                        for t, (k, is_pool) in enumerate(order):
                            if t + 2 < NT and order[t + 2][1]:
                                wtps[t + 2] = emit_relocate(
                                    oh0, hr, order[t + 2][0])
                            emit_tap(rp, prp, paccs, oh0, hr, k,
                                     wtps.pop(t, None),
                                     start=t == 0, stop=t == NT - 1)
                        emit_store(op_, paccs, oh0, hr)

    nc.finalize()
    return nc


_NC_CACHE = None


def _get_nc():
    global _NC_CACHE
    if _NC_CACHE is None:
        _NC_CACHE = _build_bass()
    return _NC_CACHE


def _prepare_in_maps(x, w_compress, w_encoder):
    x = np.asarray(x, dtype=np.float32)
    w_compress = np.asarray(w_compress, dtype=np.float32)
    w_encoder = np.asarray(w_encoder, dtype=np.float32)
    B = x.shape[0]
    assert B == NCORES

    bf = ml_dtypes.bfloat16

    # fold conv1x1 into the encoder: W[o,c,di,dj], lhsT layout [c, t*25+o]
    Wf = np.einsum("oikl,ic->ockl", w_encoder, w_compress[:, :, 0, 0])
    w9T = np.ascontiguousarray(
        Wf.transpose(1, 2, 3, 0).reshape(C, 9 * NT)
    ).astype(bf)

    ones = np.ones((NT, NT), dtype=bf)
    eye = np.eye(C, dtype=bf)

    xbf = x.astype(bf)
    # column-parity split with zero padding (pad=2 rows; pad cols in m)
    xe = np.zeros((B, C, RT, ME), dtype=bf)
    xo = np.zeros((B, C, RT, MO), dtype=bf)
    xe[:, :, 2:162, 1:81] = xbf[:, :, :, 0::2]
    xo[:, :, 2:162, 1:81] = xbf[:, :, :, 1::2]

    return [
        {
            "xe": np.ascontiguousarray(xe[b].reshape(C, RT * ME)),
            "xo": np.ascontiguousarray(xo[b].reshape(C, RT * MO)),
            "w9T": w9T,
            "ones25": ones,
            "eye128": eye,
        }
        for b in range(B)
    ]


def kernel(x, w_compress, w_encoder, **run_kwargs):
    in_maps = _prepare_in_maps(x, w_compress, w_encoder)
    nc = _get_nc()
    res = run_bass_kernel_spmd(
        nc, in_maps, core_ids=list(range(NCORES)), **run_kwargs
    )
    out = np.stack([res.results[b]["out"] for b in range(NCORES)], axis=0)
    if run_kwargs:
        kernel.last_results = res
    return out.astype(np.float32)


if __name__ == "__main__":
    rng = np.random.default_rng(0)
    x = rng.standard_normal((8, C, H, W), dtype=np.float32)
    wc = rng.standard_normal((16, C, 1, 1), dtype=np.float32) / np.sqrt(C)
    we = rng.standard_normal((NT, 16, 3, 3), dtype=np.float32) / np.sqrt(16 * 9)
    out = kernel(x, wc, we)
    print(out.shape, out.dtype)
